# revision 1
# baseline (speedup 1.0000x reference)
"""KV-cache append kernel for Trainium2 (8 NeuronCores, SPMD).

Reference semantics (B=4, H=32, L=4096, D=128, S=1, context_length=4096):
    k_new = concat(k_cache, k, axis=2)[:, :, -4096:]
    v_new = concat(v_cache, v, axis=2)[:, :, -4096:]
i.e. each (b, h) slice of the output is the cache shifted left by one
position along the sequence dim with the new token written at the end.

Implementation: pure DRAM->DRAM DMA shift-copy.  The (B, H) = 128 slices
are sharded 16-per-core across 8 NeuronCores (no cross-core traffic).
Per core, per cache: one bulk DMA moving 16 x 4095*128 f32 contiguous
chunks (~32 MB) shifted by one token, plus one small DMA writing the 16
new-token rows.  Everything is issued on the sync engine (HWDGE) and
completion is awaited with a DMA semaphore.
"""

import sys

for _p in ("/opt/trn_rl_repo",):
    if _p not in sys.path:
        sys.path.insert(0, _p)

import numpy as np

import concourse.bass as bass
import concourse.mybir as mybir
from concourse.bass_utils import run_bass_kernel_spmd

B, H, L, D = 4, 32, 4096, 128
S = 1                     # new tokens per step
NCORES = 8
BH = B * H                # 128 (b, h) slices total
SL = BH // NCORES         # 16 slices per core
ROW = L * D               # 524288 elements per slice
TOK = S * D               # 128 elements of new token per slice
BULK = (L - S) * D        # 524160 shifted elements per slice

_nc_cache = None


def _build_program():
    nc = bass.Bass("TRN2", target_bir_lowering=False)

    kc = nc.dram_tensor("k_cache", [SL, ROW], mybir.dt.float32, kind="ExternalInput")
    vc = nc.dram_tensor("v_cache", [SL, ROW], mybir.dt.float32, kind="ExternalInput")
    kt = nc.dram_tensor("k_tok", [SL, TOK], mybir.dt.float32, kind="ExternalInput")
    vt = nc.dram_tensor("v_tok", [SL, TOK], mybir.dt.float32, kind="ExternalInput")
    ko = nc.dram_tensor("k_out", [SL, ROW], mybir.dt.float32, kind="ExternalOutput")
    vo = nc.dram_tensor("v_out", [SL, ROW], mybir.dt.float32, kind="ExternalOutput")

    with nc.semaphore("dma_sem") as sem, nc.Block() as block:

        @block.sync
        def _(sync):
            # Bulk shift: out[s, 0:BULK] = cache[s, TOK:ROW] for all 16 slices.
            sync.dma_start(
                bass.AP(ko, 0, [[ROW, SL], [1, BULK]]),
                bass.AP(kc, TOK, [[ROW, SL], [1, BULK]]),
            ).then_inc(sem, 16)
            sync.dma_start(
                bass.AP(vo, 0, [[ROW, SL], [1, BULK]]),
                bass.AP(vc, TOK, [[ROW, SL], [1, BULK]]),
            ).then_inc(sem, 16)
            # Tail: out[s, BULK:ROW] = new token row s.
            sync.dma_start(
                bass.AP(ko, BULK, [[ROW, SL], [1, TOK]]),
                bass.AP(kt, 0, [[TOK, SL], [1, TOK]]),
            ).then_inc(sem, 16)
            sync.dma_start(
                bass.AP(vo, BULK, [[ROW, SL], [1, TOK]]),
                bass.AP(vt, 0, [[TOK, SL], [1, TOK]]),
            ).then_inc(sem, 16)
            sync.wait_ge(sem, 64)

    return nc


def _shard(a, row):
    """(B, H, seq, D) array -> list of NCORES contiguous (SL, row) shards."""
    a = np.ascontiguousarray(np.asarray(a), dtype=np.float32).reshape(BH, row)
    return [np.ascontiguousarray(a[c * SL : (c + 1) * SL]) for c in range(NCORES)]


def _run(k_cache, v_cache, k, v, trace=False, **spmd_kwargs):
    global _nc_cache
    if _nc_cache is None:
        _nc_cache = _build_program()

    kcs, vcs = _shard(k_cache, ROW), _shard(v_cache, ROW)
    kts, vts = _shard(k, TOK), _shard(v, TOK)
    in_maps = [
        {"k_cache": kcs[c], "v_cache": vcs[c], "k_tok": kts[c], "v_tok": vts[c]}
        for c in range(NCORES)
    ]
    res = run_bass_kernel_spmd(
        _nc_cache, in_maps, core_ids=list(range(NCORES)), trace=trace, **spmd_kwargs
    )
    k_out = np.concatenate(
        [np.asarray(res.results[c]["k_out"]) for c in range(NCORES)], axis=0
    ).reshape(B, H, L, D)
    v_out = np.concatenate(
        [np.asarray(res.results[c]["v_out"]) for c in range(NCORES)], axis=0
    ).reshape(B, H, L, D)
    return (k_out, v_out), res


def kernel(k_cache, v_cache, k, v, context_length=4096, **_ignored):
    outs, _res = _run(k_cache, v_cache, k, v, trace=False)
    return outs


# revision 2
# speedup vs baseline: 1.0833x; 1.0833x over previous
"""KV-cache append kernel for Trainium2 (8 NeuronCores, SPMD).

Reference semantics (B=4, H=32, L=4096, D=128, S=1, context_length=4096):
    k_new = concat(k_cache, k, axis=2)[:, :, -4096:]
    v_new = concat(v_cache, v, axis=2)[:, :, -4096:]
i.e. each (b, h) slice of the output is the cache shifted left by one
position along the sequence dim with the new token written at the end.

Implementation: pure DRAM->DRAM DMA shift-copy.  The (B, H) = 128 slices
are sharded 16-per-core across 8 NeuronCores (no cross-core traffic).
Per core, per cache: one bulk DMA moving 16 x 4095*128 f32 contiguous
chunks (~32 MB) shifted by one token, plus one small DMA writing the 16
new-token rows.  Everything is issued on the sync engine (HWDGE) and
completion is awaited with a DMA semaphore.
"""

import sys

for _p in ("/opt/trn_rl_repo",):
    if _p not in sys.path:
        sys.path.insert(0, _p)

import numpy as np

import concourse.bass as bass
import concourse.mybir as mybir
from concourse.bass_utils import run_bass_kernel_spmd

B, H, L, D = 4, 32, 4096, 128
S = 1                     # new tokens per step
NCORES = 8
BH = B * H                # 128 (b, h) slices total
SL = BH // NCORES         # 16 slices per core
ROW = L * D               # 524288 elements per slice
TOK = S * D               # 128 elements of new token per slice
BULK = (L - S) * D        # 524160 shifted elements per slice

_nc_cache = None


def _build_program():
    nc = bass.Bass("TRN2", target_bir_lowering=False)

    kc = nc.dram_tensor("k_cache", [SL, ROW], mybir.dt.float32, kind="ExternalInput")
    vc = nc.dram_tensor("v_cache", [SL, ROW], mybir.dt.float32, kind="ExternalInput")
    kt = nc.dram_tensor("k_tok", [SL, TOK], mybir.dt.float32, kind="ExternalInput")
    vt = nc.dram_tensor("v_tok", [SL, TOK], mybir.dt.float32, kind="ExternalInput")
    ko = nc.dram_tensor("k_out", [SL, ROW], mybir.dt.float32, kind="ExternalOutput")
    vo = nc.dram_tensor("v_out", [SL, ROW], mybir.dt.float32, kind="ExternalOutput")

    with nc.semaphore("dma_sem") as sem, nc.Block(no_gpsimd_drain=True) as block:

        @block.sync
        def _(sync):
            # Tail first (tiny, 8 KB each): out[s, BULK:ROW] = new token row s.
            # Issued ahead of the bulks so their fixed completion latency
            # hides under the bulk drain.
            sync.dma_start(
                bass.AP(ko, BULK, [[ROW, SL], [1, TOK]]),
                bass.AP(kt, 0, [[TOK, SL], [1, TOK]]),
            ).then_inc(sem, 16)
            sync.dma_start(
                bass.AP(vo, BULK, [[ROW, SL], [1, TOK]]),
                bass.AP(vt, 0, [[TOK, SL], [1, TOK]]),
            ).then_inc(sem, 16)
            # Bulk shift: out[s, 0:BULK] = cache[s, TOK:ROW] for all 16 slices.
            sync.dma_start(
                bass.AP(ko, 0, [[ROW, SL], [1, BULK]]),
                bass.AP(kc, TOK, [[ROW, SL], [1, BULK]]),
            ).then_inc(sem, 16)
            sync.dma_start(
                bass.AP(vo, 0, [[ROW, SL], [1, BULK]]),
                bass.AP(vc, TOK, [[ROW, SL], [1, BULK]]),
            ).then_inc(sem, 16)
            sync.wait_ge(sem, 64)

    return nc


def _shard(a, row):
    """(B, H, seq, D) array -> list of NCORES contiguous (SL, row) shards."""
    a = np.ascontiguousarray(np.asarray(a), dtype=np.float32).reshape(BH, row)
    return [np.ascontiguousarray(a[c * SL : (c + 1) * SL]) for c in range(NCORES)]


def _run(k_cache, v_cache, k, v, trace=False, **spmd_kwargs):
    global _nc_cache
    if _nc_cache is None:
        _nc_cache = _build_program()

    kcs, vcs = _shard(k_cache, ROW), _shard(v_cache, ROW)
    kts, vts = _shard(k, TOK), _shard(v, TOK)
    in_maps = [
        {"k_cache": kcs[c], "v_cache": vcs[c], "k_tok": kts[c], "v_tok": vts[c]}
        for c in range(NCORES)
    ]
    res = run_bass_kernel_spmd(
        _nc_cache, in_maps, core_ids=list(range(NCORES)), trace=trace, **spmd_kwargs
    )
    k_out = np.concatenate(
        [np.asarray(res.results[c]["k_out"]) for c in range(NCORES)], axis=0
    ).reshape(B, H, L, D)
    v_out = np.concatenate(
        [np.asarray(res.results[c]["v_out"]) for c in range(NCORES)], axis=0
    ).reshape(B, H, L, D)
    return (k_out, v_out), res


def kernel(k_cache, v_cache, k, v, context_length=4096, **_ignored):
    outs, _res = _run(k_cache, v_cache, k, v, trace=False)
    return outs
